# revision 1
# baseline (speedup 1.0000x reference)
"""Trainium2 Bass kernel for nn_Head_75118978007668.

Computes, for x:[B,S,D], concept_map(cm):[D,D,D] (B=4, S=2048, D=128):
    s[b,t] = sum_{j<t} lam^(t-j) x[b,j]          (lam = 1/1.2 decayed prefix sum)
    out[b,t,f] = sum_{d,e} x[b,t,d] * s[b,t,e] * cm[f,d,e]

Sharding: 8 cores, each owns 1024 contiguous positions of one batch row
(4 rows x 2 halves).  The scan carry across the half-split is recovered
exactly (to fp32) from a 256-position halo, since lam^256 ~ 4.5e-21 is far
below fp32 resolution.

Per-core dataflow (positions tiled 8 x 128):
  - carries: small PE matmuls build s(tile_start) for all 8 tiles at once
  - s tiles: triangular matmul  s = L @ x_tile + pow (x) carry   (PE, fp32)
  - main:    Y[p, (e,f)] = xT_tile.T @ W2   (PE, fp32r, N=512 chunks)
             acc[p,f]  += s[p,e] * Y[p,(e,f)]   (DVE scalar_tensor_tensor)
  where W2[d, e*128+f] = cm[f, d, e]  (host-transposed).
"""

import numpy as np

import concourse.bass as bass
import concourse.tile as tile
from concourse import bacc, mybir
from concourse.bass import ds, ts
from concourse.bass_utils import run_bass_kernel_spmd

B, S, D = 4, 2048, 128
NCORES = 8
CHUNK = S // 2          # positions per core (1024)
NT = CHUNK // 128       # position tiles per core (8)
P = 128
HALO = 256
F32 = mybir.dt.float32
F32R = mybir.dt.float32r

# match the reference's fp32 constant 1.2 exactly
LAM = 1.0 / np.float64(np.float32(1.2))

MAIN_MM_DTYPE = F32R    # flip to F32 if fp32r hw numerics are too loose

_CACHE = {}
LAST_RESULTS = None


def _host_constants():
    k = np.arange(P, dtype=np.float64)
    i = k
    # LT[i, k] = L[k, i] = lam^(k-i) for i < k   (lhsT of the triangular scan)
    LT = np.where(i[:, None] < k[None, :], LAM ** (k[None, :] - i[:, None]), 0.0)
    powv = (LAM ** k)[None, :]                      # [1, 128]
    vw = (LAM ** (P - i))[:, None]                  # [128, 1]
    j = np.arange(HALO, dtype=np.float64)           # halo weights lam^(256-j)
    hw = (LAM ** (HALO - j)).reshape(2, P).T        # [128, 2]  hw[i, u] = lam^(256-(u*128+i))
    # M9[t, jj]: c_t = sum_jj M9[t, jj] * V9[jj];  V9 = [c0, v_0..v_7]
    t = np.arange(NT, dtype=np.float64)
    M9 = np.zeros((NT, NT + 1), dtype=np.float64)
    M9[:, 0] = LAM ** (P * t)
    for tt in range(NT):
        for jj in range(tt):
            M9[tt, jj + 1] = LAM ** (P * (tt - 1 - jj))
    LT9 = M9.T                                      # [9, 8]
    f32 = np.float32
    return {
        "lt": LT.astype(f32),
        "powv": powv.astype(f32),
        "vw": vw.astype(f32),
        "hw": hw.astype(f32),
        "lt9": LT9.astype(f32),
    }


def _build_nc():
    nc = bacc.Bacc("TRN2", target_bir_lowering=False, debug=False,
                   num_devices=NCORES)
    x_d = nc.declare_dram_parameter("x", [P, NT, P], F32, isOutput=False)        # [i, t, e]
    xt_d = nc.declare_dram_parameter("xt", [P, CHUNK], MAIN_MM_DTYPE, isOutput=False)  # [d, p]
    halo_d = nc.declare_dram_parameter("halo", [P, 2, P], F32, isOutput=False)   # [i, u, e]
    w2_d = nc.declare_dram_parameter("w2", [P, P * P], MAIN_MM_DTYPE, isOutput=False)  # [d, (e,f)]
    lt_d = nc.declare_dram_parameter("lt", [P, P], F32, isOutput=False)
    pow_d = nc.declare_dram_parameter("powv", [1, P], F32, isOutput=False)
    vw_d = nc.declare_dram_parameter("vw", [P, 1], F32, isOutput=False)
    hw_d = nc.declare_dram_parameter("hw", [P, 2], F32, isOutput=False)
    lt9_d = nc.declare_dram_parameter("lt9", [NT + 1, NT], F32, isOutput=False)
    out_d = nc.declare_dram_parameter("out", [P, NT, P], F32, isOutput=True)  # [p, t, f]

    mult = mybir.AluOpType.mult
    add = mybir.AluOpType.add

    with tile.TileContext(nc) as tc:
        with tc.tile_pool(name="consts", bufs=1) as consts:
            w2_sb = [consts.tile([P, 2048], MAIN_MM_DTYPE, name=f"w2_sb{i}")
                     for i in range(8)]
            xt_sb = consts.tile([P, CHUNK], MAIN_MM_DTYPE)
            x_sb = consts.tile([P, NT, P], F32)
            halo_sb = consts.tile([P, 2, P], F32)
            lt_sb = consts.tile([P, P], F32)
            pow_sb = consts.tile([1, P], F32)
            vw_sb = consts.tile([P, 1], F32)
            hw_sb = consts.tile([P, 2], F32)
            lt9_sb = consts.tile([NT + 1, NT], F32)
            v9_sb = consts.tile([NT + 1, P], F32)
            c0_sb = consts.tile([1, P], F32)
            va_sb = consts.tile([1, 4 * P], F32)
            vb_sb = consts.tile([1, 4 * P], F32)
            c8_sb = consts.tile([NT, P], F32)
            c_all = consts.tile([1, NT * P], F32)    # [1, (t,e)] carries
            s_sb = consts.tile([P, NT, P], F32)      # [p, t, e]
            acc = consts.tile([P, NT, P], F32)       # [p, t, f]

            for i in range(8):
                nc.sync.dma_start(out=w2_sb[i][:, :],
                                  in_=w2_d[:, ds(2048 * i, 2048)])
            nc.sync.dma_start(out=xt_sb[:, :], in_=xt_d[:, :])
            nc.sync.dma_start(out=x_sb[:, :, :], in_=x_d[:, :, :])
            nc.sync.dma_start(out=halo_sb[:, :, :], in_=halo_d[:, :, :])
            nc.sync.dma_start(out=lt_sb[:, :], in_=lt_d[:, :])
            nc.sync.dma_start(out=pow_sb[:, :], in_=pow_d[:, :])
            nc.sync.dma_start(out=vw_sb[:, :], in_=vw_d[:, :])
            nc.sync.dma_start(out=hw_sb[:, :], in_=hw_d[:, :])
            nc.sync.dma_start(out=lt9_sb[:, :], in_=lt9_d[:, :])

            nc.vector.memset(acc[:, :, :], 0.0)

            # ---- carries: c_t = s[tile_start t] for all 8 tiles ----
            with tc.tile_pool(name="psum_c", bufs=1, space="PSUM") as psum_c:
                c0_ps = psum_c.tile([1, P], F32)
                nc.tensor.matmul(c0_ps[:, :], lhsT=hw_sb[:, 0:1],
                                 rhs=halo_sb[:, 0, :], start=True, stop=False)
                nc.tensor.matmul(c0_ps[:, :], lhsT=hw_sb[:, 1:2],
                                 rhs=halo_sb[:, 1, :], start=False, stop=True)
                vps_a = psum_c.tile([1, 4 * P], F32, tag="vps_a")
                vps_b = psum_c.tile([1, 4 * P], F32, tag="vps_b")
                nc.tensor.matmul(vps_a[:, :], lhsT=vw_sb[:, :],
                                 rhs=x_sb[:, 0:4, :], start=True, stop=True)
                nc.tensor.matmul(vps_b[:, :], lhsT=vw_sb[:, :],
                                 rhs=x_sb[:, 4:8, :], start=True, stop=True)
                nc.vector.tensor_copy(c0_sb[:, :], c0_ps[:, :])
                nc.vector.tensor_copy(va_sb[:, :], vps_a[:, :])
                nc.vector.tensor_copy(vb_sb[:, :], vps_b[:, :])
                nc.sync.dma_start(out=v9_sb[0:1, :], in_=c0_sb[:, :])
                nc.sync.dma_start(out=v9_sb[1:5, :], in_=va_sb[:, :])
                nc.sync.dma_start(out=v9_sb[5:9, :], in_=vb_sb[:, :])
                c_ps = psum_c.tile([NT, P], F32, tag="c_ps")
                nc.tensor.matmul(c_ps[:, :], lhsT=lt9_sb[:, :],
                                 rhs=v9_sb[:, :], start=True, stop=True)
                nc.vector.tensor_copy(c8_sb[:, :], c_ps[:, :])
                nc.sync.dma_start(out=c_all[:, :], in_=c8_sb[:, :])

            # ---- s tiles: s = L @ x_t + pow (x) c_t ----
            with tc.tile_pool(name="psum_s", bufs=2, space="PSUM") as psum_s:
                for t in range(NT):
                    sp = psum_s.tile([P, P], F32)
                    nc.tensor.matmul(sp[:, :], lhsT=lt_sb[:, :],
                                     rhs=x_sb[:, t, :], start=True, stop=False)
                    nc.tensor.matmul(sp[:, :], lhsT=pow_sb[:, :],
                                     rhs=c_all[:, ts(t, P)], start=False, stop=True)
                    nc.vector.tensor_copy(s_sb[:, t, :], sp[:, :])

            # ---- main: Y = xT_t.T @ W2 chunks; acc += s_e * Y_e ----
            with tc.tile_pool(name="psum_y", bufs=8, space="PSUM") as psum_y:
                for t in range(NT):
                    xt_t = xt_sb[:, ts(t, P)]
                    for c in range(32):
                        yp = psum_y.tile([P, 512], F32)
                        nc.tensor.matmul(
                            yp[:, :], lhsT=xt_t,
                            rhs=w2_sb[c // 4][:, ds(512 * (c % 4), 512)],
                            start=True, stop=True)
                        for jj in range(4):
                            e = 4 * c + jj
                            nc.vector.scalar_tensor_tensor(
                                out=acc[:, t, :],
                                in0=yp[:, ts(jj, P)],
                                scalar=s_sb[:, t, e:e + 1],
                                in1=acc[:, t, :],
                                op0=mult, op1=add)

            nc.sync.dma_start(out=out_d[:, :, :], in_=acc[:, :, :])
    nc.finalize()
    return nc


def _get_nc():
    if "nc" not in _CACHE:
        _CACHE["nc"] = _build_nc()
    return _CACHE["nc"]


def kernel(x, concept_map, _trace=False):
    global LAST_RESULTS
    x = np.asarray(x, dtype=np.float32)
    cm = np.asarray(concept_map, dtype=np.float32)
    assert x.shape == (B, S, D) and cm.shape == (D, D, D)

    consts = _host_constants()
    # W2[d, e*128+f] = cm[f, d, e]
    w2 = np.ascontiguousarray(np.transpose(cm, (1, 2, 0)).reshape(D, D * D))

    in_maps = []
    for core in range(NCORES):
        b, half = divmod(core, 2)
        lo = half * CHUNK
        xc = x[b, lo:lo + CHUNK]                          # [1024, 128]
        # [i, t, e] interleaved layout (partition = within-tile position)
        x_il = np.ascontiguousarray(
            xc.reshape(NT, P, D).transpose(1, 0, 2))
        xt = np.ascontiguousarray(xc.T)                   # [d, p]
        if half == 0:
            halo = np.zeros((P, 2, D), dtype=np.float32)
        else:
            h = x[b, lo - HALO:lo]                        # [256, 128]
            halo = np.ascontiguousarray(h.reshape(2, P, D).transpose(1, 0, 2))
        in_maps.append({
            "x": x_il, "xt": xt, "halo": halo, "w2": w2, **consts,
        })

    nc = _get_nc()
    res = run_bass_kernel_spmd(nc, in_maps, list(range(NCORES)), trace=_trace)
    LAST_RESULTS = res

    out = np.empty((B, S, D), dtype=np.float32)
    for core in range(NCORES):
        b, half = divmod(core, 2)
        o = res.results[core]["out"]                      # [p, t, f]
        out[b, half * CHUNK:(half + 1) * CHUNK] = (
            o.transpose(1, 0, 2).reshape(CHUNK, D))
    return out



# revision 4
# speedup vs baseline: 1.7477x; 1.7477x over previous
"""Trainium2 Bass kernel for nn_Head_75118978007668.

Computes, for x:[B,S,D], concept_map(cm):[D,D,D] (B=4, S=2048, D=128):
    s[b,t] = sum_{j<t} lam^(t-j) x[b,j]          (lam = 1/1.2 decayed prefix sum)
    out[b,t,f] = sum_{d,e} x[b,t,d] * s[b,t,e] * cm[f,d,e]

Sharding: 8 cores, each owns 1024 contiguous positions of one batch row
(4 rows x 2 halves).  The scan carry across the half-split is recovered
exactly (to fp32) from a 256-position halo, since lam^256 ~ 4.5e-21 is far
below fp32 resolution.

Per-core dataflow (positions tiled 8 x 128):
  - carries: small PE matmuls build s(tile_start) for all 8 tiles at once
  - s tiles: triangular matmul  s = L @ x_tile + pow (x) carry   (PE, fp32)
  - main:    Y[p, (e,f)] = xT_tile.T @ W2   (PE, bf16 in / fp32 psum out)
    then the weighted e-reduction acc[p,f] += s[p,e] * Y[p,(e,f)] is split
    across three engine streams (per tile of 128 e-values):
      A (56 e): DVE scalar_tensor_tensor directly from PSUM, 2 alternating
        accumulators to keep the dependency chain off the critical path.
      B (40 e): Act (scalar engine) scaled-copies z_e = s_e*Y_e into bf16
        SBUF columns; DVE folds the 40 columns with a wide bf16 add-tree.
      C (32 e): Act copies Y chunks PSUM->SBUF; GpSimd does the STT there
        (GpSimd has no PSUM port).
  where W2[d, e*128+f] = cm[f, d, e]  (host-transposed, bf16).
"""

import numpy as np
import ml_dtypes

import concourse.bass as bass
import concourse.tile as tile
from concourse import bacc, mybir
from concourse.bass import ds, ts
from concourse.bass_utils import run_bass_kernel_spmd

B, S, D = 4, 2048, 128
NCORES = 8
CHUNK = S // 2          # positions per core (1024)
NT = CHUNK // 128       # position tiles per core (8)
P = 128
HALO = 256
F32 = mybir.dt.float32
BF16 = mybir.dt.bfloat16

# match the reference's fp32 constant 1.2 exactly
LAM = 1.0 / np.float64(np.float32(1.2))

# per-tile chunk stream assignment: 32 chunks of 4 consecutive e's.
#   'A' -> DVE STT direct from PSUM into 2 alternating accumulators
#   'B' -> Act scaled-copy z_e = s_e*Y_e into bf16 z columns
#   'C' -> Act copies the Y chunk PSUM->SBUF, GpSimd broadcast-mult into
#          bf16 z columns (GpSimd has no PSUM port and no STT support)
# the z columns of B and C are folded by a wide bf16 add-tree on DVE.
def _chunk_stream(i):
    if i % 3 == 0:
        return "C"                       # 11 chunks = 44 e
    if i % 3 == 1 and i < 27:
        return "B"                       # 9 chunks = 36 e
    return "A"                           # 12 chunks = 48 e

NB = 4 * sum(1 for i in range(32) if _chunk_stream(i) == "B")   # 36
NC = 4 * sum(1 for i in range(32) if _chunk_stream(i) == "C")   # 44
ZB = NB + NC                                                    # 80 z columns

_CACHE = {}
LAST_RESULTS = None


def _host_constants():
    k = np.arange(P, dtype=np.float64)
    i = k
    # LT[i, k] = L[k, i] = lam^(k-i) for i < k   (lhsT of the triangular scan)
    LT = np.where(i[:, None] < k[None, :], LAM ** (k[None, :] - i[:, None]), 0.0)
    powv = (LAM ** k)[None, :]                      # [1, 128]
    vw = (LAM ** (P - i))[:, None]                  # [128, 1]
    j = np.arange(HALO, dtype=np.float64)           # halo weights lam^(256-j)
    hw = (LAM ** (HALO - j)).reshape(2, P).T        # [128, 2]  hw[i, u] = lam^(256-(u*128+i))
    # M9[t, jj]: c_t = sum_jj M9[t, jj] * V9[jj];  V9 = [c0, v_0..v_7]
    t = np.arange(NT, dtype=np.float64)
    M9 = np.zeros((NT, NT + 1), dtype=np.float64)
    M9[:, 0] = LAM ** (P * t)
    for tt in range(NT):
        for jj in range(tt):
            M9[tt, jj + 1] = LAM ** (P * (tt - 1 - jj))
    LT9 = M9.T                                      # [9, 8]
    f32 = np.float32
    return {
        "lt": LT.astype(f32),
        "powv": powv.astype(f32),
        "vw": vw.astype(f32),
        "hw": hw.astype(f32),
        "lt9": LT9.astype(f32),
    }


def _build_nc():
    nc = bacc.Bacc("TRN2", target_bir_lowering=False, debug=False,
                   num_devices=NCORES)
    x_d = nc.declare_dram_parameter("x", [P, NT, P], F32, isOutput=False)        # [i, t, e]
    xt_d = nc.declare_dram_parameter("xt", [P, CHUNK], BF16, isOutput=False)     # [d, p]
    halo_d = nc.declare_dram_parameter("halo", [P, 2, P], F32, isOutput=False)   # [i, u, e]
    w2_d = nc.declare_dram_parameter("w2", [P, P * P], BF16, isOutput=False)     # [d, (e,f)]
    lt_d = nc.declare_dram_parameter("lt", [P, P], F32, isOutput=False)
    pow_d = nc.declare_dram_parameter("powv", [1, P], F32, isOutput=False)
    vw_d = nc.declare_dram_parameter("vw", [P, 1], F32, isOutput=False)
    hw_d = nc.declare_dram_parameter("hw", [P, 2], F32, isOutput=False)
    lt9_d = nc.declare_dram_parameter("lt9", [NT + 1, NT], F32, isOutput=False)
    out_d = nc.declare_dram_parameter("out", [P, NT, P], F32, isOutput=True)  # [p, t, f]

    mult = mybir.AluOpType.mult
    add = mybir.AluOpType.add

    with tile.TileContext(nc) as tc:
        with tc.tile_pool(name="consts", bufs=1) as consts:
            w2_sb = [consts.tile([P, 2048], BF16, name=f"w2_sb{i}")
                     for i in range(8)]
            xt_sb = consts.tile([P, CHUNK], BF16)
            x_sb = consts.tile([P, NT, P], F32)
            halo_sb = consts.tile([P, 2, P], F32)
            lt_sb = consts.tile([P, P], F32)
            pow_sb = consts.tile([1, P], F32)
            vw_sb = consts.tile([P, 1], F32)
            hw_sb = consts.tile([P, 2], F32)
            lt9_sb = consts.tile([NT + 1, NT], F32)
            v9_sb = consts.tile([NT + 1, P], F32)
            c0_sb = consts.tile([1, P], F32)
            va_sb = consts.tile([1, 4 * P], F32)
            vb_sb = consts.tile([1, 4 * P], F32)
            c8_sb = consts.tile([NT, P], F32)
            c_all = consts.tile([1, NT * P], F32)    # [1, (t,e)] carries
            s_sb = consts.tile([P, NT, P], F32)      # [p, t, e]
            out_sb = consts.tile([P, NT, P], F32)    # [p, t, f]

            for i in range(8):
                nc.sync.dma_start(out=w2_sb[i][:, :],
                                  in_=w2_d[:, ds(2048 * i, 2048)])
            nc.sync.dma_start(out=xt_sb[:, :], in_=xt_d[:, :])
            nc.sync.dma_start(out=x_sb[:, :, :], in_=x_d[:, :, :])
            nc.sync.dma_start(out=halo_sb[:, :, :], in_=halo_d[:, :, :])
            nc.sync.dma_start(out=lt_sb[:, :], in_=lt_d[:, :])
            nc.sync.dma_start(out=pow_sb[:, :], in_=pow_d[:, :])
            nc.sync.dma_start(out=vw_sb[:, :], in_=vw_d[:, :])
            nc.sync.dma_start(out=hw_sb[:, :], in_=hw_d[:, :])
            nc.sync.dma_start(out=lt9_sb[:, :], in_=lt9_d[:, :])

            # ---- carries: c_t = s[tile_start t] for all 8 tiles ----
            with tc.tile_pool(name="psum_c", bufs=1, space="PSUM") as psum_c:
                c0_ps = psum_c.tile([1, P], F32)
                nc.tensor.matmul(c0_ps[:, :], lhsT=hw_sb[:, 0:1],
                                 rhs=halo_sb[:, 0, :], start=True, stop=False)
                nc.tensor.matmul(c0_ps[:, :], lhsT=hw_sb[:, 1:2],
                                 rhs=halo_sb[:, 1, :], start=False, stop=True)
                vps_a = psum_c.tile([1, 4 * P], F32, tag="vps_a")
                vps_b = psum_c.tile([1, 4 * P], F32, tag="vps_b")
                nc.tensor.matmul(vps_a[:, :], lhsT=vw_sb[:, :],
                                 rhs=x_sb[:, 0:4, :], start=True, stop=True)
                nc.tensor.matmul(vps_b[:, :], lhsT=vw_sb[:, :],
                                 rhs=x_sb[:, 4:8, :], start=True, stop=True)
                nc.vector.tensor_copy(c0_sb[:, :], c0_ps[:, :])
                nc.vector.tensor_copy(va_sb[:, :], vps_a[:, :])
                nc.vector.tensor_copy(vb_sb[:, :], vps_b[:, :])
                nc.sync.dma_start(out=v9_sb[0:1, :], in_=c0_sb[:, :])
                nc.sync.dma_start(out=v9_sb[1:5, :], in_=va_sb[:, :])
                nc.sync.dma_start(out=v9_sb[5:9, :], in_=vb_sb[:, :])
                c_ps = psum_c.tile([NT, P], F32, tag="c_ps")
                nc.tensor.matmul(c_ps[:, :], lhsT=lt9_sb[:, :],
                                 rhs=v9_sb[:, :], start=True, stop=True)
                nc.vector.tensor_copy(c8_sb[:, :], c_ps[:, :])
                nc.sync.dma_start(out=c_all[:, :], in_=c8_sb[:, :])

            # ---- s tiles: s = L @ x_t + pow (x) c_t ----
            with tc.tile_pool(name="psum_s", bufs=2, space="PSUM") as psum_s:
                for t in range(NT):
                    sp = psum_s.tile([P, P], F32)
                    nc.tensor.matmul(sp[:, :], lhsT=lt_sb[:, :],
                                     rhs=x_sb[:, t, :], start=True, stop=False)
                    nc.tensor.matmul(sp[:, :], lhsT=pow_sb[:, :],
                                     rhs=c_all[:, ts(t, P)], start=False, stop=True)
                    nc.vector.tensor_copy(s_sb[:, t, :], sp[:, :])

            # ---- main: Y = xT_t.T @ W2 chunks; 3-stream weighted e-reduce ----
            with tc.tile_pool(name="psum_y", bufs=8, space="PSUM") as psum_y, \
                 tc.tile_pool(name="zpool", bufs=2) as zpool, \
                 tc.tile_pool(name="ypool", bufs=3) as ypool, \
                 tc.tile_pool(name="apool", bufs=2) as apool:
                for t in range(NT):
                    xt_t = xt_sb[:, ts(t, P)]
                    zb = zpool.tile([P, ZB, P], BF16)
                    accA = apool.tile([P, 2, P], F32)
                    nc.vector.memset(accA[:, :, :], 0.0)
                    ka = kb = 0
                    kc = NB
                    for c in range(32):
                        stream = _chunk_stream(c)
                        yp = psum_y.tile([P, 512], F32)
                        nc.tensor.matmul(
                            yp[:, :], lhsT=xt_t,
                            rhs=w2_sb[c // 4][:, ds(512 * (c % 4), 512)],
                            start=True, stop=True)
                        if stream == "A":
                            for jj in range(4):
                                e = 4 * c + jj
                                nc.vector.scalar_tensor_tensor(
                                    out=accA[:, ka % 2, :],
                                    in0=yp[:, ts(jj, P)],
                                    scalar=s_sb[:, t, e:e + 1],
                                    in1=accA[:, ka % 2, :],
                                    op0=mult, op1=add)
                                ka += 1
                        elif stream == "B":
                            for jj in range(4):
                                e = 4 * c + jj
                                nc.scalar.mul(zb[:, kb, :], yp[:, ts(jj, P)],
                                              s_sb[:, t, e:e + 1])
                                kb += 1
                        else:  # "C"
                            ysb = ypool.tile([P, 512], F32)
                            nc.scalar.copy(ysb[:, :], yp[:, :])
                            for jj in range(4):
                                e = 4 * c + jj
                                nc.gpsimd.tensor_tensor(
                                    zb[:, kc, :], ysb[:, ts(jj, P)],
                                    s_sb[:, t, e:e + 1].to_broadcast([P, P]),
                                    mult)
                                kc += 1

                    # fold the 80 bf16 z columns: 80=40+40 ->20 ->10 ->5 ->2+2+1
                    nc.vector.tensor_tensor(zb[:, 0:40, :], zb[:, 0:40, :],
                                            zb[:, 40:80, :], add)
                    nc.vector.tensor_tensor(zb[:, 0:20, :], zb[:, 0:20, :],
                                            zb[:, 20:40, :], add)
                    nc.vector.tensor_tensor(zb[:, 0:10, :], zb[:, 0:10, :],
                                            zb[:, 10:20, :], add)
                    nc.vector.tensor_tensor(zb[:, 0:5, :], zb[:, 0:5, :],
                                            zb[:, 5:10, :], add)
                    nc.vector.tensor_tensor(zb[:, 0:2, :], zb[:, 0:2, :],
                                            zb[:, 2:4, :], add)
                    nc.vector.tensor_tensor(zb[:, 0, :], zb[:, 0, :],
                                            zb[:, 1, :], add)
                    nc.vector.tensor_tensor(zb[:, 0, :], zb[:, 0, :],
                                            zb[:, 4, :], add)
                    # combine: out = (A0+A1) + z
                    nc.vector.tensor_tensor(accA[:, 0, :], accA[:, 0, :],
                                            accA[:, 1, :], add)
                    nc.vector.tensor_tensor(out_sb[:, t, :], accA[:, 0, :],
                                            zb[:, 0, :], add)
                    nc.sync.dma_start(out=out_d[:, t, :], in_=out_sb[:, t, :])
    nc.finalize()
    return nc


def _get_nc():
    if "nc" not in _CACHE:
        _CACHE["nc"] = _build_nc()
    return _CACHE["nc"]


def kernel(x, concept_map, _trace=False):
    global LAST_RESULTS
    x = np.asarray(x, dtype=np.float32)
    cm = np.asarray(concept_map, dtype=np.float32)
    assert x.shape == (B, S, D) and cm.shape == (D, D, D)

    consts = _host_constants()
    # W2[d, e*128+f] = cm[f, d, e]
    w2 = np.ascontiguousarray(
        np.transpose(cm, (1, 2, 0)).reshape(D, D * D)).astype(ml_dtypes.bfloat16)

    in_maps = []
    for core in range(NCORES):
        b, half = divmod(core, 2)
        lo = half * CHUNK
        xc = x[b, lo:lo + CHUNK]                          # [1024, 128]
        # [i, t, e] interleaved layout (partition = within-tile position)
        x_il = np.ascontiguousarray(
            xc.reshape(NT, P, D).transpose(1, 0, 2))
        xt = np.ascontiguousarray(xc.T).astype(ml_dtypes.bfloat16)  # [d, p]
        if half == 0:
            halo = np.zeros((P, 2, D), dtype=np.float32)
        else:
            h = x[b, lo - HALO:lo]                        # [256, 128]
            halo = np.ascontiguousarray(h.reshape(2, P, D).transpose(1, 0, 2))
        in_maps.append({
            "x": x_il, "xt": xt, "halo": halo, "w2": w2, **consts,
        })

    nc = _get_nc()
    res = run_bass_kernel_spmd(nc, in_maps, list(range(NCORES)), trace=_trace)
    LAST_RESULTS = res

    out = np.empty((B, S, D), dtype=np.float32)
    for core in range(NCORES):
        b, half = divmod(core, 2)
        o = res.results[core]["out"]                      # [p, t, f]
        out[b, half * CHUNK:(half + 1) * CHUNK] = (
            o.transpose(1, 0, 2).reshape(CHUNK, D))
    return out


# revision 7
# speedup vs baseline: 2.0229x; 1.1574x over previous
"""Trainium2 Bass kernel for nn_Head_75118978007668.

Computes, for x:[B,S,D], concept_map(cm):[D,D,D] (B=4, S=2048, D=128):
    s[b,t] = sum_{j<t} lam^(t-j) x[b,j]          (lam = 1/1.2 decayed prefix sum)
    out[b,t,f] = sum_{d,e} x[b,t,d] * s[b,t,e] * cm[f,d,e]

Sharding: 8 cores, each owns 1024 contiguous positions of one batch row
(4 rows x 2 halves).  The scan carry across the half-split is recovered
exactly (to fp32) from a 256-position halo, since lam^256 ~ 4.5e-21 is far
below fp32 resolution.

Per-core dataflow (positions tiled 8 x 128):
  - carries: small PE matmuls build s(tile_start) for all 8 tiles at once
  - s tiles: triangular matmul  s = L @ x_tile + pow (x) carry   (PE, fp32)
  - main:    Y[p, (e,f)] = xT_tile.T @ W2   (PE, bf16 in / fp32 psum out)
    then the weighted e-reduction acc[p,f] += s[p,e] * Y[p,(e,f)] is split
    across three engine streams (per tile of 128 e-values):
      A (56 e): DVE scalar_tensor_tensor directly from PSUM, 2 alternating
        accumulators to keep the dependency chain off the critical path.
      B (40 e): Act (scalar engine) scaled-copies z_e = s_e*Y_e into bf16
        SBUF columns; DVE folds the 40 columns with a wide bf16 add-tree.
      C (32 e): Act copies Y chunks PSUM->SBUF; GpSimd does the STT there
        (GpSimd has no PSUM port).
  where W2[d, e*128+f] = cm[f, d, e]  (host-transposed, bf16).
"""

import numpy as np
import ml_dtypes

import concourse.bass as bass
import concourse.tile as tile
from concourse import bacc, mybir
from concourse.bass import ds, ts
from concourse.bass_utils import run_bass_kernel_spmd

B, S, D = 4, 2048, 128
NCORES = 8
CHUNK = S // 2          # positions per core (1024)
NT = CHUNK // 128       # position tiles per core (8)
P = 128
HALO = 256
F32 = mybir.dt.float32
BF16 = mybir.dt.bfloat16

# match the reference's fp32 constant 1.2 exactly
LAM = 1.0 / np.float64(np.float32(1.2))

# Every e produces one bf16 z column z_e = s_e * Y_e (z col index == e);
# a 7-level bf16 add-tree on DVE folds all 128 columns into the output.
# Producers per tile, by chunk group g (chunks 4g..4g+3, e 16g..16g+15):
#   chunk 4g   ('C'): Act copies Y chunk PSUM->SBUF, GpSimd broadcast-mult
#                     (GpSimd has no PSUM port, ~600ns/e, stream-bound)
#   4g+1,4g+2  ('D'): one DVE broadcast-mult over a [P,1024] 2-bank PSUM
#                     mega tile = 8 e per op (amortizes DVE fixed cost)
#   chunk 4g+3 ('B'): Act per-e scaled-copy for g<6, else 'C'
ZB = 128

_CACHE = {}
LAST_RESULTS = None


def _host_constants():
    k = np.arange(P, dtype=np.float64)
    i = k
    # LT[i, k] = L[k, i] = lam^(k-i) for i < k   (lhsT of the triangular scan)
    LT = np.where(i[:, None] < k[None, :], LAM ** (k[None, :] - i[:, None]), 0.0)
    powv = (LAM ** k)[None, :]                      # [1, 128]
    vw = (LAM ** (P - i))[:, None]                  # [128, 1]
    j = np.arange(HALO, dtype=np.float64)           # halo weights lam^(256-j)
    hw = (LAM ** (HALO - j)).reshape(2, P).T        # [128, 2]  hw[i, u] = lam^(256-(u*128+i))
    # M9[t, jj]: c_t = sum_jj M9[t, jj] * V9[jj];  V9 = [c0, v_0..v_7]
    t = np.arange(NT, dtype=np.float64)
    M9 = np.zeros((NT, NT + 1), dtype=np.float64)
    M9[:, 0] = LAM ** (P * t)
    for tt in range(NT):
        for jj in range(tt):
            M9[tt, jj + 1] = LAM ** (P * (tt - 1 - jj))
    LT9 = M9.T                                      # [9, 8]
    f32 = np.float32
    return {
        "lt": LT.astype(f32),
        "powv": powv.astype(f32),
        "vw": vw.astype(f32),
        "hw": hw.astype(f32),
        "lt9": LT9.astype(f32),
    }


def _build_nc():
    nc = bacc.Bacc("TRN2", target_bir_lowering=False, debug=False,
                   num_devices=NCORES)
    x_d = nc.declare_dram_parameter("x", [P, NT, P], F32, isOutput=False)        # [i, t, e]
    xt_d = nc.declare_dram_parameter("xt", [P, CHUNK], BF16, isOutput=False)     # [d, p]
    halo_d = nc.declare_dram_parameter("halo", [P, 2, P], F32, isOutput=False)   # [i, u, e]
    w2_d = nc.declare_dram_parameter("w2", [P, P * P], BF16, isOutput=False)     # [d, (e,f)]
    lt_d = nc.declare_dram_parameter("lt", [P, P], F32, isOutput=False)
    pow_d = nc.declare_dram_parameter("powv", [1, P], F32, isOutput=False)
    vw_d = nc.declare_dram_parameter("vw", [P, 1], F32, isOutput=False)
    hw_d = nc.declare_dram_parameter("hw", [P, 2], F32, isOutput=False)
    lt9_d = nc.declare_dram_parameter("lt9", [NT + 1, NT], F32, isOutput=False)
    out_d = nc.declare_dram_parameter("out", [P, NT, P], F32, isOutput=True)  # [p, t, f]

    mult = mybir.AluOpType.mult
    add = mybir.AluOpType.add

    with tile.TileContext(nc) as tc:
        with tc.tile_pool(name="consts", bufs=1) as consts:
            w2_sb = [consts.tile([P, 2048], BF16, name=f"w2_sb{i}")
                     for i in range(8)]
            xt_sb = consts.tile([P, CHUNK], BF16)
            x_sb = consts.tile([P, NT, P], F32)
            halo_sb = consts.tile([P, 2, P], F32)
            lt_sb = consts.tile([P, P], F32)
            pow_sb = consts.tile([1, P], F32)
            vw_sb = consts.tile([P, 1], F32)
            hw_sb = consts.tile([P, 2], F32)
            lt9_sb = consts.tile([NT + 1, NT], F32)
            v9_sb = consts.tile([NT + 1, P], F32)
            c0_sb = consts.tile([1, P], F32)
            va_sb = consts.tile([1, 4 * P], F32)
            vb_sb = consts.tile([1, 4 * P], F32)
            c8_sb = consts.tile([NT, P], F32)
            c_all = consts.tile([1, NT * P], F32)    # [1, (t,e)] carries
            s_sb = consts.tile([P, NT, P], F32)      # [p, t, e]
            out_sb = consts.tile([P, NT, P], F32)    # [p, t, f]

            for i in range(8):
                nc.sync.dma_start(out=w2_sb[i][:, :],
                                  in_=w2_d[:, ds(2048 * i, 2048)])
            nc.sync.dma_start(out=xt_sb[:, :], in_=xt_d[:, :])
            nc.sync.dma_start(out=x_sb[:, :, :], in_=x_d[:, :, :])
            nc.sync.dma_start(out=halo_sb[:, :, :], in_=halo_d[:, :, :])
            nc.sync.dma_start(out=lt_sb[:, :], in_=lt_d[:, :])
            nc.sync.dma_start(out=pow_sb[:, :], in_=pow_d[:, :])
            nc.sync.dma_start(out=vw_sb[:, :], in_=vw_d[:, :])
            nc.sync.dma_start(out=hw_sb[:, :], in_=hw_d[:, :])
            nc.sync.dma_start(out=lt9_sb[:, :], in_=lt9_d[:, :])

            # ---- carries: c_t = s[tile_start t] for all 8 tiles ----
            with tc.tile_pool(name="psum_c", bufs=1, space="PSUM") as psum_c:
                c0_ps = psum_c.tile([1, P], F32)
                nc.tensor.matmul(c0_ps[:, :], lhsT=hw_sb[:, 0:1],
                                 rhs=halo_sb[:, 0, :], start=True, stop=False)
                nc.tensor.matmul(c0_ps[:, :], lhsT=hw_sb[:, 1:2],
                                 rhs=halo_sb[:, 1, :], start=False, stop=True)
                vps_a = psum_c.tile([1, 4 * P], F32, tag="vps_a")
                vps_b = psum_c.tile([1, 4 * P], F32, tag="vps_b")
                nc.tensor.matmul(vps_a[:, :], lhsT=vw_sb[:, :],
                                 rhs=x_sb[:, 0:4, :], start=True, stop=True)
                nc.tensor.matmul(vps_b[:, :], lhsT=vw_sb[:, :],
                                 rhs=x_sb[:, 4:8, :], start=True, stop=True)
                nc.vector.tensor_copy(c0_sb[:, :], c0_ps[:, :])
                nc.vector.tensor_copy(va_sb[:, :], vps_a[:, :])
                nc.vector.tensor_copy(vb_sb[:, :], vps_b[:, :])
                nc.sync.dma_start(out=v9_sb[0:1, :], in_=c0_sb[:, :])
                nc.sync.dma_start(out=v9_sb[1:5, :], in_=va_sb[:, :])
                nc.sync.dma_start(out=v9_sb[5:9, :], in_=vb_sb[:, :])
                c_ps = psum_c.tile([NT, P], F32, tag="c_ps")
                nc.tensor.matmul(c_ps[:, :], lhsT=lt9_sb[:, :],
                                 rhs=v9_sb[:, :], start=True, stop=True)
                nc.vector.tensor_copy(c8_sb[:, :], c_ps[:, :])
                nc.sync.dma_start(out=c_all[:, :], in_=c8_sb[:, :])

            # ---- s tiles: s = L @ x_t + pow (x) c_t ----
            with tc.tile_pool(name="psum_s", bufs=2, space="PSUM") as psum_s:
                for t in range(NT):
                    sp = psum_s.tile([P, P], F32)
                    nc.tensor.matmul(sp[:, :], lhsT=lt_sb[:, :],
                                     rhs=x_sb[:, t, :], start=True, stop=False)
                    nc.tensor.matmul(sp[:, :], lhsT=pow_sb[:, :],
                                     rhs=c_all[:, ts(t, P)], start=False, stop=True)
                    nc.vector.tensor_copy(s_sb[:, t, :], sp[:, :])

            # ---- main: Y = xT_t.T @ W2 chunks; 3-stream weighted e-reduce ----
            with tc.tile_pool(name="psum_y", bufs=4, space="PSUM") as psum_y, \
                 tc.tile_pool(name="psum_m", bufs=2, space="PSUM") as psum_m, \
                 tc.tile_pool(name="zpool", bufs=2) as zpool, \
                 tc.tile_pool(name="ypool", bufs=3) as ypool:
                for t in range(NT):
                    xt_t = xt_sb[:, ts(t, P)]
                    zb = zpool.tile([P, ZB, P], BF16)
                    for g in range(8):
                        # ---- 'C' chunk 4g: Act copy + GpSimd per-e mult ----
                        c = 4 * g
                        yp = psum_y.tile([P, 512], F32)
                        nc.tensor.matmul(
                            yp[:, :], lhsT=xt_t,
                            rhs=w2_sb[c // 4][:, ds(512 * (c % 4), 512)],
                            start=True, stop=True)
                        ysb = ypool.tile([P, 512], F32)
                        nc.scalar.copy(ysb[:, :], yp[:, :])
                        for jj in range(4):
                            e = 4 * c + jj
                            nc.gpsimd.tensor_tensor(
                                zb[:, e, :], ysb[:, ts(jj, P)],
                                s_sb[:, t, e:e + 1].to_broadcast([P, P]),
                                mult)
                        # ---- 'D' chunks 4g+1, 4g+2: DVE 8-e mega mult ----
                        mp = psum_m.tile([P, 8, P], F32)
                        for h in range(2):
                            c = 4 * g + 1 + h
                            nc.tensor.matmul(
                                mp[:, 4 * h:4 * h + 4, :], lhsT=xt_t,
                                rhs=w2_sb[c // 4][:, ds(512 * (c % 4), 512)],
                                start=True, stop=True)
                        e0 = 4 * (4 * g + 1)
                        nc.vector.tensor_tensor(
                            zb[:, e0:e0 + 8, :],
                            mp[:, :, :],
                            s_sb[:, t, e0:e0 + 8, None].to_broadcast([P, 8, P]),
                            mult)
                        # ---- chunk 4g+3: 'B' (Act per-e z) for g<6 else 'C'
                        c = 4 * g + 3
                        yp = psum_y.tile([P, 512], F32)
                        nc.tensor.matmul(
                            yp[:, :], lhsT=xt_t,
                            rhs=w2_sb[c // 4][:, ds(512 * (c % 4), 512)],
                            start=True, stop=True)
                        if g < 6:
                            for jj in range(4):
                                e = 4 * c + jj
                                nc.scalar.mul(zb[:, e, :], yp[:, ts(jj, P)],
                                              s_sb[:, t, e:e + 1])
                        else:
                            ysb = ypool.tile([P, 512], F32)
                            nc.scalar.copy(ysb[:, :], yp[:, :])
                            for jj in range(4):
                                e = 4 * c + jj
                                nc.gpsimd.tensor_tensor(
                                    zb[:, e, :], ysb[:, ts(jj, P)],
                                    s_sb[:, t, e:e + 1].to_broadcast([P, P]),
                                    mult)

                    # fold 128 bf16 z columns: 7 halving levels, last into f32
                    for half in (64, 32, 16, 8, 4, 2):
                        nc.vector.tensor_tensor(
                            zb[:, 0:half, :], zb[:, 0:half, :],
                            zb[:, half:2 * half, :], add)
                    nc.vector.tensor_tensor(out_sb[:, t, :], zb[:, 0, :],
                                            zb[:, 1, :], add)
                    nc.sync.dma_start(out=out_d[:, t, :], in_=out_sb[:, t, :])
    nc.finalize()
    return nc


def _get_nc():
    if "nc" not in _CACHE:
        _CACHE["nc"] = _build_nc()
    return _CACHE["nc"]


def kernel(x, concept_map, _trace=False):
    global LAST_RESULTS
    x = np.asarray(x, dtype=np.float32)
    cm = np.asarray(concept_map, dtype=np.float32)
    assert x.shape == (B, S, D) and cm.shape == (D, D, D)

    consts = _host_constants()
    # W2[d, e*128+f] = cm[f, d, e]
    w2 = np.ascontiguousarray(
        np.transpose(cm, (1, 2, 0)).reshape(D, D * D)).astype(ml_dtypes.bfloat16)

    in_maps = []
    for core in range(NCORES):
        b, half = divmod(core, 2)
        lo = half * CHUNK
        xc = x[b, lo:lo + CHUNK]                          # [1024, 128]
        # [i, t, e] interleaved layout (partition = within-tile position)
        x_il = np.ascontiguousarray(
            xc.reshape(NT, P, D).transpose(1, 0, 2))
        xt = np.ascontiguousarray(xc.T).astype(ml_dtypes.bfloat16)  # [d, p]
        if half == 0:
            halo = np.zeros((P, 2, D), dtype=np.float32)
        else:
            h = x[b, lo - HALO:lo]                        # [256, 128]
            halo = np.ascontiguousarray(h.reshape(2, P, D).transpose(1, 0, 2))
        in_maps.append({
            "x": x_il, "xt": xt, "halo": halo, "w2": w2, **consts,
        })

    nc = _get_nc()
    res = run_bass_kernel_spmd(nc, in_maps, list(range(NCORES)), trace=_trace)
    LAST_RESULTS = res

    out = np.empty((B, S, D), dtype=np.float32)
    for core in range(NCORES):
        b, half = divmod(core, 2)
        o = res.results[core]["out"]                      # [p, t, f]
        out[b, half * CHUNK:(half + 1) * CHUNK] = (
            o.transpose(1, 0, 2).reshape(CHUNK, D))
    return out
